# revision 31
# baseline (speedup 1.0000x reference)
"""AdaptiveRankTensorizedLinear (CP, rank 64) forward on 8 TRN2 NeuronCores.

Math: with A = KhatriRao(U1,U2,U3) (4096x64), B = KhatriRao(V1,V2,V3) (4096x64),
    y = (x @ A) @ (lam*B)^T + bias
Data-parallel over the 4096-token batch: each core handles 512 rows of x.

x crosses the DRAM interface in float8_e3m4 (pure host dtype cast; the PE
matmul takes the fp8 rhs against a bf16 stationary A directly, so no on-chip
up-cast pass is needed).  y returns in bf16.  ~3.5 MB/core of HBM traffic.
Quantization study on the seed-0 inputs: e3m4-x + bf16 factors/y = 1.44%
rel err vs the 2e-2 gate (measured in CoreSim end-to-end).

Schedule notes (every engine/DMA choice below was measured from NTFF
profiles of prior revisions):
  - DMAs are few and big: HWDGE issue costs ~0.7us of engine time per
    dma_start and sub-256KB pieces are overhead-dominated.  Loads use all
    three queues (sync/scalar HWDGE + gpsimd SWDGE for x-t0's second half).
  - fac is laid out V-block first and its first piece is split across both
    HWDGE rings, so the tiny scalars the BT build needs (V1T/V2T/V3T/lam)
    land first and the A quarters (DVE tensor_mul, ~0.45us each) pace mm1
    of tile 0 right as x tile 0 arrives.
  - BT_aug is built as 16 per-o1 tensor_scalar expansions (scalar =
    per-partition f32 V1 column) split ACT(0-5)/DVE(6-15, after A) — a
    broadcast-AP tensor_tensor runs ~1 elem/lane/cycle with a per-subtile
    bubble (4x slower), and gpsimd tensor_scalar on [64,256] measured ~4us
    per op, so gpsimd gets none.
  - PSUM->SBUF evacs run at 1x (f32 source); they are paired into
    [128,1024] ops (amortizing the ~120-cycle PSUM overhead) aligned 1:1
    with 0.25MB store quarters (q0,q2->DVE, q1,q3->ACT; the last tile
    splits each evac across both engines to shorten the drain); psy pool =
    3 x 2-bank tiles.
  - Four 128-row tiles pipeline load -> mm1 -> mm2 -> evac -> store; store
    quarters go sync/gpsimd/scalar/sync.  mm1 of tile t+1 is emitted
    between mm2 pairs 1 and 2 of tile t so psy-recycle waits are stale by
    the time the PE reaches them.
  - 16 N=256 dummy matmuls release the HAM clock gate (one full 3.4us busy
    window) and 12 more keep the PE busy through worst-case DMA arrival so
    a late load cannot re-throttle the gate (exec_time is the max over 8
    cores, each with an independent free-running HAM window phase — the
    fillers buy determinism, not just speed).  A dummy ACT op pre-triggers
    the activation table load.

Per-tile PE work: mm1 = 32 accumulating matmuls (A chunk [128,64] bf16
stationary x x-chunk [128,128] e3m4 moving) -> t [64,128] PSUM; bf16 copy
into tt (row 64 = ones); mm2 = 8 matmuls (tt stationary x BT_aug [65,512]
moving, row 64 adds bias) -> f32 PSUM.
"""

import numpy as np
import ml_dtypes

BF16 = ml_dtypes.bfloat16
E3M4 = ml_dtypes.float8_e3m4

NCORES = 8
B_TOTAL = 4096
B_SHARD = B_TOTAL // NCORES  # 512
IN = 4096
OUT = 4096
D = 16
R = 64

T_TILE = 128
N_TILES = B_SHARD // T_TILE  # 4
KCHUNK = 128
N_KCHUNKS = IN // KCHUNK  # 32

# fac packed layout (bf16, [128, FAC_W]) — V block first so the BT build
# can start from the first (smallest) DMA piece:
#   [0:64, 0:16]     V1T[r, o] = V1[o, r]
#   [0:64, 16:32]    V2T ; [0:64, 32:48] V3T
#   [0:64, 48:49]    lam
#   [:, 64:128]      U3rep[p, r]  = U3[p % 16, r]
#   [:, 128:256]     U2rep[p, h*64+r] = U2[8h + p//16, r]
#   [:, 256:1280]    U1bc[p, i*64+r]  = U1[i, r]   (broadcast to all p)
FAC_W = 1280

_CACHE = {}


def _build_nc():
    from contextlib import ExitStack

    from concourse import bacc, mybir
    import concourse.tile as tile

    f32 = mybir.dt.float32
    bf16 = mybir.dt.bfloat16
    f8e3 = mybir.dt.float8e3

    nc = bacc.Bacc(None, target_bir_lowering=False)

    x_ext = nc.declare_dram_parameter(
        "x", [N_TILES, KCHUNK, N_KCHUNKS * T_TILE], f8e3, isOutput=False
    )
    fac_ext = nc.declare_dram_parameter("fac", [128, FAC_W], bf16, isOutput=False)
    bias_ext = nc.declare_dram_parameter("bias", [OUT], f32, isOutput=False)
    out_ext = nc.declare_dram_parameter(
        "out", [N_TILES, KCHUNK, OUT], bf16, isOutput=True
    )

    with tile.TileContext(nc) as tc, ExitStack() as ctx:
        const = ctx.enter_context(tc.tile_pool(name="const", bufs=1))
        y_pool = ctx.enter_context(tc.tile_pool(name="y", bufs=4))
        pst_pool = ctx.enter_context(tc.tile_pool(name="pst", bufs=2, space="PSUM"))
        psy_pool = ctx.enter_context(tc.tile_pool(name="psy", bufs=3, space="PSUM"))

        # ---- loads: V/U3/U2 block first on sync; U1bc halves on scalar ------
        fac = const.tile([128, FAC_W], bf16)
        nc.sync.dma_start(out=fac[:, 0:128], in_=fac_ext[:, 0:128])
        nc.scalar.dma_start(out=fac[:, 128:256], in_=fac_ext[:, 128:256])
        nc.scalar.dma_start(out=fac[:, 256:768], in_=fac_ext[:, 256:768])
        nc.scalar.dma_start(out=fac[:, 768:1280], in_=fac_ext[:, 768:1280])

        W = N_KCHUNKS * T_TILE  # 4096
        x_tiles = []
        for t in range(N_TILES):
            x_sb = const.tile([KCHUNK, W], f8e3, tag=f"x{t}")
            x_tiles.append(x_sb)
        # three load queues: sync (fac1, t0a, t2), scalar (U1bc, t1b, t3),
        # gpsimd SWDGE (bias, t0b, t1a) — gpsimd is otherwise idle early and
        # removes the 2.1us PE stall on the second half of tile 0
        nc.sync.dma_start(out=x_tiles[0][:, 0:2048], in_=x_ext[0, :, 0:2048])
        nc.sync.dma_start(out=x_tiles[1][:, 0:2048], in_=x_ext[1, :, 0:2048])
        nc.scalar.dma_start(out=x_tiles[1][:, 2048:4096], in_=x_ext[1, :, 2048:4096])
        nc.sync.dma_start(out=x_tiles[2][:], in_=x_ext[2])
        nc.scalar.dma_start(out=x_tiles[3][:], in_=x_ext[3])

        # ---- DVE memsets, then PE warm-up matmuls --------------------------
        warm_sb = const.tile([128, 512], bf16)
        nc.vector.memset(warm_sb[:], 0.0)
        t_aug = []
        for i in range(2):
            t = const.tile([R + 1, T_TILE], bf16, tag=f"t_aug{i}")
            nc.vector.memset(t[R : R + 1, :], 1.0)
            t_aug.append(t)
        act_dummy = const.tile([R, D], bf16)
        nc.vector.memset(act_dummy[:], 0.0)
        # pre-trigger the ACT activation-table load (otherwise it serializes
        # in front of the first real ACT op mid-kernel)
        nc.scalar.copy(act_dummy[:], act_dummy[:])
        # 9 x 427ns(cold) = 3.8us of sustained PE activity: the HAM clock
        # gate needs one full 3.4us busy window to release BEFORE mm1 starts
        # (a shorter burst, or one interrupted by a load stall, leaves the
        # whole first half of the kernel at 1.2GHz)
        # 16 N=256 warmups (3.4us cold) release the HAM clock gate; 12 more
        # filler matmuls keep the PE busy through the worst-case A/x arrival
        # (~2.5us later) so a late DMA cannot re-throttle the gate before
        # mm1 starts.  On fast runs the fillers cost ~1.3us (warm rate) but
        # guarantee mm1 runs at 2.4GHz.
        ps_warm = psy_pool.tile([128, 1024], f32, tag="ps_y")
        for i in range(16):
            nc.tensor.matmul(
                ps_warm[:, 0:256], warm_sb[:, 0:128], warm_sb[:, 0:256],
                start=(i == 0), stop=(i == 15),
            )
        for i in range(12):
            nc.tensor.matmul(
                ps_warm[:, 256:512], warm_sb[:, 0:128], warm_sb[:, 0:256],
                start=(i == 0), stop=(i == 11),
            )

        V1T = fac[0:R, 0:16]
        V2T = fac[0:R, 16:32]
        V3T = fac[0:R, 32:48]
        lamT = fac[0:R, 48:49]
        U3rep = fac[:, 64:128]
        U2rep = fac[:, 128:256]
        U1bc = fac[:, 256:1280]

        # f32 copies of the tensor_scalar scalar operands (AP scalars must
        # be f32); on DVE early so ACT's BT ops are not blocked
        V1Tf = const.tile([R, D], f32)
        nc.vector.tensor_copy(V1Tf, V1T)
        lamf = const.tile([R, 1], f32)
        nc.vector.tensor_copy(lamf, lamT)

        # B23[p, 64g+r] = U2rep[p, 64g+r] * U3rep[p, r]   (DVE)
        B23 = const.tile([128, 2 * R], bf16)
        nc.vector.tensor_mul(
            B23[:].rearrange("p (h r) -> p h r", h=2),
            U2rep.rearrange("p (h r) -> p h r", h=2),
            U3rep.unsqueeze(1).broadcast_to([128, 2, R]),
        )
        # A chunks: A_sb[p, 64c + r] = U1[c//2, r] * B23[p, 64*(c%2) + r]
        # built in 4 column-quarters that pace mm1 of tile 0
        A_sb = const.tile([128, N_KCHUNKS * R], bf16)
        for q in range(4):
            nc.vector.tensor_mul(
                A_sb[:, q * 512 : (q + 1) * 512].rearrange(
                    "p (i g r) -> p i g r", i=4, g=2
                ),
                U1bc[:, q * 256 : (q + 1) * 256]
                .rearrange("p (i r) -> p i r", i=4)
                .unsqueeze(2)
                .broadcast_to([128, 4, 2, R]),
                B23[:].rearrange("p (g r) -> p g r", g=2)
                .unsqueeze(1)
                .broadcast_to([128, 4, 2, R]),
            )

        # ---- BT_aug build ---------------------------------------------------
        # BT_aug[r, 256*o1 + 16*o2 + o3] = lam[r]*V1[o1,r]*V2[o2,r]*V3[o3,r]
        BT_aug = const.tile([R + 1, OUT], bf16)
        # bias row via SWDGE cast-DMA, issued first on gpsimd, then the
        # second half of x tile 0 on the same (otherwise idle) queue
        nc.gpsimd.dma_start(out=BT_aug[R : R + 1, :], in_=bias_ext[:].unsqueeze(0))
        nc.gpsimd.dma_start(out=x_tiles[0][:, 2048:4096], in_=x_ext[0, :, 2048:4096])
        # gpsimd: V2Ts = lam*V2T, then W23[r, 16*o2+o3] = V2Ts[r,o2]*V3T[r,o3]
        V2Ts = const.tile([R, D], bf16)
        nc.gpsimd.tensor_scalar_mul(V2Ts, V2T, lamf)
        W23 = const.tile([R, D * D], bf16)
        nc.gpsimd.tensor_mul(
            W23[:].rearrange("p (a b) -> p a b", a=16),
            V2Ts[:].unsqueeze(2).broadcast_to([R, D, D]),
            V3T.unsqueeze(1).broadcast_to([R, D, D]),
        )
        # 16 per-o1 expansions: ACT o1 0-5, DVE (after A) 6-15.  gpsimd gets
        # none — its tensor_scalar on [64,256] measured ~4us per op.
        for o1 in range(D):
            dst = BT_aug[0:R, o1 * 256 : (o1 + 1) * 256]
            sc = V1Tf[:, o1 : o1 + 1]
            if o1 < 6:
                nc.scalar.mul(dst, W23[:], sc)
            else:
                nc.vector.tensor_scalar_mul(dst, W23[:], sc)

        # stores: quarter q of tile t -> queue; evac engine per quarter
        # (q0,q2 -> DVE; q1,q3 -> ACT; gpsimd cannot read PSUM).  Each
        # 0.25MB store piece waits on exactly one paired [128,1024] evac.
        store_eng = (nc.sync, nc.gpsimd, nc.scalar, nc.sync)

        # ---------------- main loop: four 128-row tiles ----------------------
        # PE stream: mm1(t+1) is emitted between mm2 pairs 1 and 2 of tile t
        # so the psy-recycle wait (pair j needs the evac of pair j-3) is ~2us
        # stale by the time it is checked, instead of stalling ~1us per tile.
        def mm1(t):
            x_sb = x_tiles[t]
            ps_t = pst_pool.tile([R, T_TILE], f32, name="ps_t")
            for c in range(N_KCHUNKS):
                nc.tensor.matmul(
                    ps_t[:],
                    A_sb[:, c * R : (c + 1) * R],
                    x_sb[:, c * T_TILE : (c + 1) * T_TILE],
                    start=(c == 0),
                    stop=(c == N_KCHUNKS - 1),
                )
            tt = t_aug[t % 2]
            nc.vector.tensor_copy(tt[0:R, :], ps_t[:])
            return tt

        def mm2_pair(t, tt, y_sb, q):
            ps_y = psy_pool.tile([T_TILE, 1024], f32, tag="ps_y", name="ps_y")
            for h in range(2):
                n = 2 * q + h
                nc.tensor.matmul(
                    ps_y[:, h * 512 : (h + 1) * 512],
                    tt[:],
                    BT_aug[:, n * 512 : (n + 1) * 512],
                    start=True,
                    stop=True,
                )
            w = OUT // 4
            dst = y_sb[:, q * w : (q + 1) * w]
            if t == N_TILES - 1:
                nc.vector.tensor_copy(dst[:, 0:512], ps_y[:, 0:512])
                nc.scalar.copy(dst[:, 512:1024], ps_y[:, 512:1024])
            elif q % 2 == 0:
                nc.vector.tensor_copy(dst, ps_y[:])
            else:
                nc.scalar.copy(dst, ps_y[:])
            if t == N_TILES - 1 and q >= 2:
                if q == 2:
                    # keep the scalar ring clear for the final quarter
                    nc.gpsimd.dma_start(
                        out=out_ext[t, :, q * w : (q + 1) * w], in_=dst
                    )
                else:
                    # final quarter: two 128KB pieces on the two (now idle)
                    # HWDGE rings, each gated on a single evac engine, so the
                    # kernel's last HBM write completes ~1us sooner
                    nc.sync.dma_start(
                        out=out_ext[t, :, 3 * w : 3 * w + 512],
                        in_=dst[:, 0:512],
                    )
                    nc.scalar.dma_start(
                        out=out_ext[t, :, 3 * w + 512 : OUT],
                        in_=dst[:, 512:1024],
                    )
            else:
                store_eng[q].dma_start(
                    out=out_ext[t, :, q * w : (q + 1) * w],
                    in_=dst,
                )

        tts = {0: mm1(0)}
        y_sbs = {}
        for t in range(N_TILES):
            y_sbs[t] = y_pool.tile([T_TILE, OUT], bf16, tag="y", name="y_sb")
            mm2_pair(t, tts[t], y_sbs[t], 0)
            mm2_pair(t, tts[t], y_sbs[t], 1)
            if t + 1 < N_TILES:
                tts[t + 1] = mm1(t + 1)
            mm2_pair(t, tts[t], y_sbs[t], 2)
            mm2_pair(t, tts[t], y_sbs[t], 3)

    nc.compile()
    return nc


def _get_nc():
    if "nc" not in _CACHE:
        _CACHE["nc"] = _build_nc()
    return _CACHE["nc"]


def _prep_x_shards(x):
    """Cast x to float8_e3m4 and block-transpose: per core i, tile t,
    shard[t, p, c*128 + b] = x[i*512 + t*128 + b, c*128 + p]."""
    xq = np.asarray(x, dtype=np.float32).astype(E3M4)
    xr = xq.reshape(NCORES, N_TILES, T_TILE, N_KCHUNKS, KCHUNK).transpose(
        0, 1, 4, 3, 2
    )
    xr = np.ascontiguousarray(xr).reshape(
        NCORES, N_TILES, KCHUNK, N_KCHUNKS * T_TILE
    )
    return [xr[i] for i in range(NCORES)]


def _prep_fac(U1, U2, U3, V1, V2, V3, lam):
    """Pack factor replications/layouts (no arithmetic) into one bf16 array."""
    fac = np.zeros((128, FAC_W), dtype=BF16)
    fac[0:R, 0:16] = np.asarray(V1, np.float32).T.astype(BF16)
    fac[0:R, 16:32] = np.asarray(V2, np.float32).T.astype(BF16)
    fac[0:R, 32:48] = np.asarray(V3, np.float32).T.astype(BF16)
    fac[0:R, 48] = np.asarray(lam, np.float32).astype(BF16)
    fac[:, 64:128] = np.tile(np.asarray(U3, np.float32), (8, 1)).astype(BF16)
    U2f = np.asarray(U2, np.float32)
    for h in range(2):
        # U2rep[p, h*64+r] = U2[8h + p//16, r]
        fac[:, 128 + h * 64 : 192 + h * 64] = np.repeat(
            U2f[8 * h : 8 * h + 8], 16, axis=0
        ).astype(BF16)
    fac[:, 256:1280] = np.broadcast_to(
        np.asarray(U1, np.float32).reshape(1, 1024), (128, 1024)
    ).astype(BF16)
    return fac


def kernel(x, U1, U2, U3, V1, V2, V3, lam, bias):
    from concourse.bass_utils import run_bass_kernel_spmd

    nc = _get_nc()

    shards = _prep_x_shards(x)
    fac = _prep_fac(U1, U2, U3, V1, V2, V3, lam)
    bias_f = np.ascontiguousarray(np.asarray(bias, dtype=np.float32))

    in_maps = [
        {"x": shards[i], "fac": fac, "bias": bias_f} for i in range(NCORES)
    ]
    res = run_bass_kernel_spmd(nc, in_maps, core_ids=list(range(NCORES)))
    _CACHE["last_results"] = res
    out = np.concatenate(
        [
            np.asarray(res.results[i]["out"]).reshape(B_SHARD, OUT)
            for i in range(NCORES)
        ],
        axis=0,
    )
    return out.astype(np.float32)


def last_exec_time_ns():
    res = _CACHE.get("last_results")
    return None if res is None else res.exec_time_ns
